# revision 6
# baseline (speedup 1.0000x reference)
"""Trainium2 Bass kernel for the span-search problem (nn_DCR_21285857919673).

Data-parallel over batch: 32 batches / 8 cores = 4 per core; batches globally
sorted by span into 4 width slots. Host ships seq pre-transposed ([h, token]),
compacted to the valid token span, as fp16 hi + fp8e4(lo*2^13) (3B/elem vs
fp32's 4). Per h-chunk the PE streams 3W rows: one 4-col fp16 pass
(hi x [q1h q2h q1l q2l] -> PSUM rows 0:4), one mixed-dtype pass
(lo8 x fp16(qh*2^-13) subnormal weights, accumulating rows 0:2 -- the 2^13
scales cancel exactly), and one fp16 ones-matmul over hi^2 for n2.

The drain Act-copies PSUM [4,W]+[1,W] to SBUF and DMAs 5 rows to DRAM
scratch. The banded stage re-reads them token-major-per-partition
(partition p holds tokens m*p..m*p+m+31, an [[m,128],[1,m+32]] gather), so
windows become overlapping free-dim APs and the d1/d2 pair-adds run on
39-wide tiles. Validity masks are gone entirely: host pads data columns
with -q2 (pad sims land >=0.3 below any real sim) and the scratch tail
[W, SP) is memset to d=-1e6 / n2=512.
"""
import sys

sys.path.insert(0, "/opt/trn_rl_repo")

import numpy as np
import ml_dtypes

import concourse.bass as bass
import concourse.bacc as bacc
import concourse.bass_isa as bass_isa
import concourse.mybir as mybir
import concourse.tile as tile
from concourse.alu_op_type import AluOpType
from concourse.bass_utils import run_bass_kernel_spmd

F32 = mybir.dt.float32
F16 = mybir.dt.float16
F8E4 = mybir.dt.float8e4

B = 32
S = 1024
H = 1024
L = 32
NC = H // 128
NCORES = 8
NEG = -10000.0
LOSC = 2.0 ** 13

HI_GROUPS = {
    0: [[0], [1], [2, 3], [4, 5], [6, 7]],
    1: [[0], [1], [2, 3], [4, 5], [6, 7]],
    2: [[0], [1], [2, 3], [4, 5], [6, 7]],
    3: [[0], [1], [2, 3], [4, 5], [6, 7]],
}
LO_GROUPS = {
    0: [[0, 1, 2, 3], [4, 5, 6, 7]],
    1: [[0, 1, 2, 3], [4, 5, 6, 7]],
    2: [[0, 1, 2, 3], [4, 5, 6, 7]],
    3: [[0, 1, 2, 3], [4, 5, 6, 7]],
}

_cache = {}


def _pieces(w):
    return [(off, min(512, w - off)) for off in range(0, w, 512)]


def _build(W, NT):
    M_MAX = max(NT)
    SP = 128 * M_MAX + 40
    LC = sum(NT)
    nc = bacc.Bacc("TRN2", target_bir_lowering=False, debug=False)

    his = [nc.dram_tensor(f"hi{k}", [128, NC * W[k]], F16,
                          kind="ExternalInput").ap() for k in range(4)]
    los = [nc.dram_tensor(f"lo{k}", [128, NC * W[k]], F8E4,
                          kind="ExternalInput").ap() for k in range(4)]
    # per (chunk c, slot k): 6 cols [q1h q2h q1l q2l q1s q2s]
    qw_in = nc.dram_tensor("qw", [128, NC * 4 * 6 + 2], F16, kind="ExternalInput").ap()
    fpack_in = nc.dram_tensor("fpack", [128, M_MAX * L + LC + 4 * 16], F32,
                              kind="ExternalInput").ap()

    mvei_out = nc.dram_tensor("mvei", [4, 2 * M_MAX * 128], F32,
                              kind="ExternalOutput").ap()
    scratch = nc.dram_tensor("scratch", [4, 5, SP], F32).ap()

    with tile.TileContext(nc) as tc:
        with (
            tc.tile_pool(name="consts", bufs=1) as consts,
            tc.tile_pool(name="seqp", bufs=1) as seqp,
            tc.tile_pool(name="sqp", bufs=8) as sqp,
            tc.tile_pool(name="rows", bufs=4) as rows_p,
            tc.tile_pool(name="band", bufs=4) as band_p,
            tc.tile_pool(name="outp", bufs=4) as out_p,
            tc.tile_pool(name="pd", bufs=2, space="PSUM") as pd,
            tc.tile_pool(name="pn", bufs=2, space="PSUM") as pn,
        ):
            # qw gates the first matmul: ship it first (tiny)
            c_qw = consts.tile([128, NC * 4 * 6 + 2], F16, tag="qw")
            nc.sync.dma_start(c_qw[:], qw_in)

            hi_tiles = {}
            lo_tiles = {}

            def emit_hi_dmas(k, gidx):
                g = HI_GROUPS[k][gidx]
                w = W[k]
                t = seqp.tile([128, len(g) * w], F16, tag=f"hi{k}_{gidx}",
                              name=f"hi_s{k}_g{g[0]}")
                nc.sync.dma_start(t[:], his[k][:, g[0] * w:(g[-1] + 1) * w])
                for gi, c in enumerate(g):
                    hi_tiles[(k, c)] = (t, gi)

            def emit_lo_dmas(k, gidx):
                g = LO_GROUPS[k][gidx]
                w = W[k]
                t = seqp.tile([128, len(g) * w], F8E4, tag=f"lo{k}_{gidx}",
                              name=f"lo_s{k}_g{g[0]}")
                nc.sync.dma_start(t[:], los[k][:, g[0] * w:(g[-1] + 1) * w])
                for gi, c in enumerate(g):
                    lo_tiles[(k, c)] = (t, gi)

            emit_hi_dmas(0, 0)
            emit_hi_dmas(0, 1)
            emit_lo_dmas(0, 0)

            c_fpack = consts.tile([128, M_MAX * L + LC + 4 * 16], F32, tag="fpack")
            nc.sync.dma_start(c_fpack[:], fpack_in)

            emit_hi_dmas(0, 2)
            emit_lo_dmas(0, 1)
            emit_hi_dmas(0, 3)
            emit_hi_dmas(0, 4)
            for k in (1, 2, 3):
                ng_hi, ng_lo = len(HI_GROUPS[k]), len(LO_GROUPS[k])
                emit_hi_dmas(k, 0)
                emit_lo_dmas(k, 0)
                for gi in range(1, ng_hi):
                    emit_hi_dmas(k, gi)
                    if gi == 1 and ng_lo > 1:
                        emit_lo_dmas(k, 1)

            # const views
            riota = c_fpack[:, 0:M_MAX * L]
            off_f = M_MAX * L
            cconst = []
            for k in range(4):
                cconst.append(c_fpack[:, off_f:off_f + NT[k]])
                off_f += NT[k]
            qcat = c_fpack[:, off_f:off_f + 4 * 16]

            qpart = consts.tile([128, 4], F32, tag="qpart")
            qtrash = consts.tile([128, 16], F32, tag="qtrash")
            qn2all = consts.tile([128, 4], F32, tag="qn2all")
            qsq = consts.tile([128, 4], F32, tag="qsq")
            rsqall = consts.tile([128, 4], F32, tag="rsqall")

            def emit_qn2():
                for k in range(4):
                    nc.scalar.activation(qtrash[:], qcat[:, k * 16:(k + 1) * 16],
                                         mybir.ActivationFunctionType.Square,
                                         accum_out=qpart[:, k:k + 1])
                nc.gpsimd.partition_all_reduce(qn2all[:], qpart[:], 128,
                                               bass_isa.ReduceOp.add)
                nc.scalar.sqrt(qsq[:], qn2all[:])
                nc.vector.reciprocal(rsqall[:], qsq[:])

            def ap3(t, m, n, lstr):
                """[128, m, L] view of tile t: free strides [n per t-step, lstr]."""
                a = t[:]
                return bass.AP(t.tensor, a.offset,
                               [[a.ap[0][0], 128], [n, m], [lstr, L]])

            def emit_phase_a(k, mid=None):
                w = W[k]
                pieces = _pieces(w)
                dps = pd.tile([4, w], F32, tag="dps", name=f"dps{k}")
                nps = pn.tile([1, w], F32, tag="nps", name=f"nps{k}")
                sq_tiles = {}

                def emit_n2_mm(c):
                    for off, ln in pieces:
                        nc.tensor.matmul(nps[0:1, off:off + ln],
                                         lhsT=c_qw[:, NC * 4 * 6:NC * 4 * 6 + 1],
                                         rhs=sq_tiles[c][:, off:off + ln],
                                         start=(c == 0), stop=(c == NC - 1))

                for c in range(NC):
                    th, gih = hi_tiles[(k, c)]
                    tl, gil = lo_tiles[(k, c)]
                    hi_c = th[:, gih * w:(gih + 1) * w]
                    lo_c = tl[:, gil * w:(gil + 1) * w]
                    qbase = (c * 4 + k) * 6
                    q4 = c_qw[:, qbase:qbase + 4]
                    q2s = c_qw[:, qbase + 4:qbase + 6]
                    for off, ln in pieces:
                        nc.tensor.matmul(dps[0:4, off:off + ln], lhsT=q4,
                                         rhs=hi_c[:, off:off + ln],
                                         start=(c == 0), stop=False,
                                         skip_group_check=True)
                    for off, ln in pieces:
                        nc.tensor.matmul(dps[0:2, off:off + ln], lhsT=q2s,
                                         rhs=lo_c[:, off:off + ln],
                                         start=False, stop=(c == NC - 1),
                                         skip_group_check=True)
                    # squares split across DVE/Act to halve per-engine load
                    sq_c = sqp.tile([128, w], F16, tag="sq", name=f"sq{k}_{c}")
                    if c % 2 == 0:
                        nc.vector.tensor_tensor(out=sq_c[:], in0=hi_c, in1=hi_c,
                                                op=AluOpType.mult)
                    else:
                        nc.scalar.activation(sq_c[:], hi_c,
                                             mybir.ActivationFunctionType.Square)
                    sq_tiles[c] = sq_c
                    # n2 matmuls lag one chunk so a late square never blocks
                    # the d-stream in the PE queue
                    if 0 < c:
                        emit_n2_mm(c - 1)
                    if c == NC - 1:
                        emit_n2_mm(c)
                    if k == 0 and c == NC - 1:
                        emit_qn2()
                    if mid is not None and c in (0, 1, 3):
                        mid(c)
                return dps, nps

            def emit_drain(k, dps, nps, last=False):
                w = W[k]
                m = NT[k]
                spw = 128 * m + 40
                base = k * 5 * SP
                dsb = rows_p.tile([4, spw], F32, tag="dsb", name=f"dsb{k}")
                n2sb = rows_p.tile([1, spw], F32, tag="n2sb", name=f"n2sb{k}")
                if last:
                    # n2 path first: its gather opens the banded chain
                    nc.vector.memset(n2sb[:, w:spw], 512.0)
                    nc.vector.memset(dsb[:, w:spw], -1e6)
                    nc.scalar.copy(n2sb[:, 0:w], nps[:])
                    nc.scalar.dma_start(
                        bass.AP(scratch.tensor, base + 4 * SP, [[1, 1], [1, spw]]),
                        n2sb[:])
                    nc.scalar.copy(dsb[:, 0:w], dps[:])
                    nc.scalar.dma_start(
                        bass.AP(scratch.tensor, base, [[SP, 4], [1, spw]]),
                        dsb[:])
                else:
                    nc.scalar.copy(dsb[:, 0:w], dps[:])
                    nc.vector.memset(dsb[:, w:spw], -1e6)
                    nc.scalar.copy(n2sb[:, 0:w], nps[:])
                    nc.vector.memset(n2sb[:, w:spw], 512.0)
                    nc.scalar.dma_start(
                        bass.AP(scratch.tensor, base, [[SP, 4], [1, spw]]),
                        dsb[:])
                    nc.scalar.dma_start(
                        bass.AP(scratch.tensor, base + 4 * SP, [[1, 1], [1, spw]]),
                        n2sb[:])

            def emit_gather(k, split=False):
                m = NT[k]
                mw = m + 32
                base = k * 5 * SP
                g4 = band_p.tile([128, 5 * mw], F32, tag="g4", name=f"g4_{k}")
                a4 = g4[:]
                if split:
                    # last slot: n2 row first (chain-start), d rows second
                    nc.sync.dma_start(
                        bass.AP(g4.tensor, a4.offset + 4 * mw,
                                [[a4.ap[0][0], 128], [1, mw]]),
                        bass.AP(scratch.tensor, base + 4 * SP,
                                [[m, 128], [1, mw]]))
                    nc.sync.dma_start(
                        bass.AP(g4.tensor, a4.offset,
                                [[a4.ap[0][0], 128], [mw, 4], [1, mw]]),
                        bass.AP(scratch.tensor, base, [[m, 128], [SP, 4], [1, mw]]))
                else:
                    nc.sync.dma_start(
                        bass.AP(g4.tensor, a4.offset,
                                [[a4.ap[0][0], 128], [mw, 5], [1, mw]]),
                        bass.AP(scratch.tensor, base, [[m, 128], [SP, 5], [1, mw]]))
                return g4

            def emit_banded1(k, g4):
                m = NT[k]
                mw = m + 32
                d1t = band_p.tile([128, m], F32, tag="d1t", name=f"d1t{k}")
                nc.vector.tensor_tensor(out=d1t[:], in0=g4[:, 0:m],
                                        in1=g4[:, 2 * mw:2 * mw + m],
                                        op=AluOpType.add)
                d2t = band_p.tile([128, mw], F32, tag="d2t", name=f"d2t{k}")
                nc.vector.tensor_tensor(out=d2t[:], in0=g4[:, mw:2 * mw],
                                        in1=g4[:, 3 * mw:4 * mw],
                                        op=AluOpType.add)
                numer = band_p.tile([128, m, L], F32, tag="numer", name=f"nu{k}")
                nc.gpsimd.tensor_tensor(
                    out=numer[:], in0=ap3(d2t, m, 1, 1),
                    in1=bass.AP(d1t.tensor, d1t[:].offset,
                                [[d1t[:].ap[0][0], 128], [1, m], [0, L]]),
                    op=AluOpType.add)
                a4 = g4[:]
                n2o = a4.offset + 4 * mw
                nsum = band_p.tile([128, m, L], F32, tag="nsum", name=f"ns{k}")
                nc.vector.tensor_tensor(
                    out=nsum[:],
                    in0=bass.AP(g4.tensor, n2o, [[a4.ap[0][0], 128], [1, m], [1, L]]),
                    in1=bass.AP(g4.tensor, n2o, [[a4.ap[0][0], 128], [1, m], [0, L]]),
                    op=AluOpType.add)
                return numer, nsum

            def emit_banded2(k, numer, nsum):
                m = NT[k]
                den = band_p.tile([128, m, L], F32, tag="den", name=f"de{k}")
                nc.scalar.sqrt(den[:], nsum[:])
                rec = band_p.tile([128, m, L], F32, tag="rec", name=f"re{k}")
                nc.vector.reciprocal(rec[:], den[:])
                sim = band_p.tile([128, m, L], F32, tag="sim", name=f"si{k}")
                nc.vector.tensor_tensor(out=sim[:], in0=numer[:], in1=rec[:],
                                        op=AluOpType.mult)
                maxv = band_p.tile([128, m], F32, tag="maxv", name=f"mx{k}")
                nc.vector.tensor_reduce(out=maxv[:], in_=sim[:],
                                        axis=mybir.AxisListType.X, op=AluOpType.max)
                eq = band_p.tile([128, m, L], F32, tag="eq", name=f"eq{k}")
                nc.vector.tensor_tensor(
                    out=eq[:], in0=sim[:],
                    in1=bass.AP(maxv.tensor, maxv[:].offset,
                                [[maxv[:].ap[0][0], 128], [1, m], [0, L]]),
                    op=AluOpType.is_equal)
                wt = band_p.tile([128, m, L], F32, tag="wt", name=f"wq{k}")
                nc.vector.tensor_tensor(
                    out=wt[:], in0=eq[:],
                    in1=bass.AP(c_fpack.tensor, riota.offset,
                                [[riota.ap[0][0], 128], [L, m], [1, L]]),
                    op=AluOpType.mult)
                mval = band_p.tile([128, m], F32, tag="mval", name=f"mv{k}")
                nc.vector.tensor_reduce(out=mval[:], in_=wt[:],
                                        axis=mybir.AxisListType.X, op=AluOpType.max)
                mvei = out_p.tile([128, 2 * m], F32, tag="mvei", name=f"me{k}")
                nc.gpsimd.tensor_tensor(out=mvei[:, m:2 * m], in0=cconst[k],
                                        in1=mval[:], op=AluOpType.subtract)
                nc.vector.tensor_scalar(out=mvei[:, 0:m], in0=maxv[:],
                                        scalar1=rsqall[:, k:k + 1], scalar2=None,
                                        op0=AluOpType.mult)
                ostate[k] = mvei
                emit_out(k)

            def emit_out(k):
                m = NT[k]
                nc.sync.dma_start(
                    bass.AP(mvei_out.tensor, k * 2 * M_MAX * 128,
                            [[2 * m, 128], [1, 2 * m]]), ostate[k][:])

            state = {}
            bstate = {}

            gstate = {}
            ostate = {}

            def mk_mid(k):
                def mid(c):
                    if c == 0:
                        emit_drain(k, *state[k])
                    elif c == 1:
                        bstate[k] = emit_banded1(k, emit_gather(k))
                    elif c == 3 and k < 2:
                        emit_banded2(k, *bstate[k])
                return mid

            for k in range(4):
                state[k] = emit_phase_a(k, mid=mk_mid(k - 1) if k > 0 else None)
            emit_drain(3, *state[3])
            g3 = emit_gather(3)
            emit_banded2(2, *bstate[2])
            emit_banded2(3, *emit_banded1(3, g3))

    nc.compile()
    return nc


def _prep_core(seq, idx, order_c, W, NT):
    M_MAX = max(NT)
    LC = sum(NT)
    im = {}
    spans = {}
    qw = np.zeros((128, NC * 4 * 6 + 2), np.float16)
    qw[:, NC * 4 * 6:] = 1.0
    fpack = np.zeros((128, M_MAX * L + LC + 4 * 16), np.float32)
    fpack[:, 0:M_MAX * L] = np.broadcast_to(
        (L - np.arange(L))[None, None, :], (128, M_MAX, L)).reshape(128, M_MAX * L)
    off_f = M_MAX * L
    for k in range(4):
        w = W[k]
        m = NT[k]
        b = order_c[k]
        sep0, sep1 = int(idx[b, 0]), int(idx[b, 1])
        span = max(0, sep1 - sep0 - 1)
        spans[k] = span
        q1 = seq[b, 1, :]
        q2 = seq[b, max(sep0 - 1, 0), :]
        q1h = q1.astype(np.float16)
        q2h = q2.astype(np.float16)
        q1l = (q1 - q1h.astype(np.float32)).astype(np.float16)
        q2l = (q2 - q2h.astype(np.float32)).astype(np.float16)
        q1s = (q1h.astype(np.float32) / LOSC).astype(np.float16)
        q2s = (q2h.astype(np.float32) / LOSC).astype(np.float16)

        x = np.ascontiguousarray(seq[b, sep0 + 1:sep0 + 1 + span, :].T)  # [H, span]
        xh = x.astype(np.float16)
        xl8 = ((x - xh.astype(np.float32)) * LOSC).astype(ml_dtypes.float8_e4m3)
        hi = np.empty((NC, 128, w), np.float16)
        hi[:, :, 0:span] = xh.reshape(NC, 128, span)
        # pad columns = -q2h (gives pad sims <= real max - 0.3)
        hi[:, :, span:] = (-q2h).reshape(NC, 128)[:, :, None]
        lo = np.zeros((NC, 128, w), ml_dtypes.float8_e4m3)
        lo[:, :, 0:span] = xl8.reshape(NC, 128, span)
        im[f"hi{k}"] = np.ascontiguousarray(
            hi.transpose(1, 0, 2).reshape(128, NC * w))
        im[f"lo{k}"] = np.ascontiguousarray(
            lo.transpose(1, 0, 2).reshape(128, NC * w)).view(np.uint8)

        for c in range(NC):
            sl = slice(c * 128, (c + 1) * 128)
            base = (c * 4 + k) * 6
            qw[:, base + 0] = q1h[sl]
            qw[:, base + 1] = q2h[sl]
            qw[:, base + 2] = q1l[sl]
            qw[:, base + 3] = q2l[sl]
            qw[:, base + 4] = q1s[sl]
            qw[:, base + 5] = q2s[sl]
        # cconst[p, t] = sep0 + 1 + (m*p + t) + L
        i_tok = np.arange(128)[:, None] * m + np.arange(m)[None, :]
        fpack[:, off_f:off_f + m] = (sep0 + 1 + i_tok + L).astype(np.float32)
        off_f += m
        fpack[:, M_MAX * L + LC + k * 16:M_MAX * L + LC + k * 16 + 8] = \
            q1.reshape(128, 8, order="F")
        fpack[:, M_MAX * L + LC + k * 16 + 8:M_MAX * L + LC + k * 16 + 16] = \
            q2.reshape(128, 8, order="F")
    im["qw"] = qw
    im["fpack"] = fpack
    return im, spans


def kernel(sequence_outputs, idxs, max_ans_len):
    seq = np.asarray(sequence_outputs, dtype=np.float32)
    idx = np.asarray(idxs).astype(np.int64)
    assert int(max_ans_len) == L and seq.shape == (B, S, H)

    spans_all = np.maximum(idx[:, 1] - idx[:, 0] - 1, 0)
    order = np.argsort(-spans_all, kind="stable")
    W = [max(2, (int(spans_all[order[k * NCORES]]) + 1) & ~1) for k in range(4)]
    NT = [(w + 127) // 128 for w in W]

    key = (tuple(W),)
    if key not in _cache:
        _cache[key] = _build(W, NT)
    nc = _cache[key]

    M_MAX = max(NT)
    in_maps, span_list = [], []
    for c in range(NCORES):
        order_c = [int(order[k * NCORES + c]) for k in range(4)]
        im, spans = _prep_core(seq, idx, order_c, W, NT)
        in_maps.append(im)
        span_list.append((order_c, spans))

    res = run_bass_kernel_spmd(nc, in_maps, core_ids=list(range(NCORES))).results

    mv = np.full((B, S), NEG, np.float32)
    ei = np.full((B, S), -1, np.int32)
    for c in range(NCORES):
        order_c, spans = span_list[c]
        for k in range(4):
            b = order_c[k]
            sep0 = int(idx[b, 0])
            span = spans[k]
            if span <= 0:
                continue
            m = NT[k]
            flat = res[c]["mvei"][k, 0:128 * 2 * m].reshape(128, 2 * m)
            mvd = flat[:, 0:m].ravel()
            eid = flat[:, m:2 * m].ravel()
            mv[b, sep0 + 1:sep0 + 1 + span] = mvd[0:span]
            ei[b, sep0 + 1:sep0 + 1 + span] = np.rint(eid[0:span]).astype(np.int32)
    return mv, ei
